# revision 1
# baseline (speedup 1.0000x reference)
"""Trainium2 Bass kernel for capsule-network AgreementRouting (n_iterations=1).

Reference computation (see problem):
    c = softmax(b, axis=-1)                  # [IN, OUT] (same for every batch)
    s[b,o,d] = sum_in c[in,o] * u[b,in,o,d]  # weighted reduce over input caps
    v = squash(s)                            # per (b,o): s * l2/(1+l2)/sqrt(l2)
    out = v[:, None]                         # [B, 1, OUT, DIM]

Strategy: data-parallel over batch across 8 NeuronCores (64 batches/core).
Per core the 47 MB u-shard is streamed through SBUF with large contiguous
DMAs; the in-caps reduction is done on TensorE with softmax(b) chunks as the
stationary operand (out[o', (g,o,d)] = sum_in c[in,o'] u[in,g,o,d]), and the
o'==o diagonal is extracted with a precomputed identity mask + strided
reduce on VectorE. Squash runs on ACT/DVE over the tiny [10, 64, 16] result.
"""

import numpy as np

import concourse.bass as bass
import concourse.tile as tile
from concourse import bacc, mybir
from concourse.bass_utils import run_bass_kernel_spmd

F32 = mybir.dt.float32
F32R = mybir.dt.float32r

B, IN_CAPS, OUT_CAPS, OUT_DIM = 512, 1152, 10, 16
N_CORES = 8
B_LOCAL = B // N_CORES            # 64 batches per core
OD = OUT_CAPS * OUT_DIM           # 160
P = 128                           # partitions
N_CHUNKS = IN_CAPS // P           # 9 contraction chunks
GROUP = 3                         # batches per PSUM accumulation group (480 cols)
TILE_B = 6                        # batches per DMA tile (~4.4 MB per DMA)
BLK = 18                          # max batches per squash/output block


def _build_core_program() -> bass.Bass:
    nc = bacc.Bacc(None)
    u = nc.dram_tensor("u", [B_LOCAL, IN_CAPS, OUT_CAPS, OUT_DIM], F32,
                       kind="ExternalInput")
    bp = nc.dram_tensor("b", [IN_CAPS, OUT_CAPS], F32, kind="ExternalInput")
    v = nc.dram_tensor("v", [OUT_CAPS, B_LOCAL, OUT_DIM], F32,
                       kind="ExternalOutput")

    # in-cap index mapping: in = p*N_CHUNKS + n (partition-major). Per (p, b)
    # the 9 chunk rows are contiguous in HBM -> 5760B runs per partition for
    # u and a single 360B run for b, keeping each DMA on one descriptor lane.
    u_r = u[:].rearrange("b (p n) o d -> p b n (o d)", p=P)
    b_r = bp[:].rearrange("(p n) o -> p n o", p=P)

    with tile.TileContext(nc) as tc:
        with (
            tc.tile_pool(name="singles", bufs=1) as singles,
            tc.tile_pool(name="inp", bufs=4) as inp,
            tc.tile_pool(name="psum", bufs=4, space="PSUM") as psum,
            tc.tile_pool(name="mids", bufs=4) as mids,
            tc.tile_pool(name="blocks", bufs=2) as blocks,
        ):
            # ---- softmax over b rows: c[in, o] ----
            b_sb = singles.tile([P, N_CHUNKS, OUT_CAPS], F32)
            nc.scalar.dma_start(out=b_sb, in_=b_r)
            bmax = singles.tile([P, N_CHUNKS], F32)
            nc.vector.reduce_max(out=bmax, in_=b_sb, axis=mybir.AxisListType.X)
            negmax = singles.tile([P, N_CHUNKS], F32)
            nc.scalar.mul(out=negmax, in_=bmax, mul=-1.0)
            e_sb = singles.tile([P, N_CHUNKS, OUT_CAPS], F32)
            for n in range(N_CHUNKS):
                nc.scalar.activation(
                    out=e_sb[:, n, :], in_=b_sb[:, n, :],
                    func=mybir.ActivationFunctionType.Exp,
                    bias=negmax[:, n : n + 1], scale=1.0,
                )
            esum = singles.tile([P, N_CHUNKS], F32)
            nc.vector.reduce_sum(out=esum, in_=e_sb, axis=mybir.AxisListType.X)
            einv = singles.tile([P, N_CHUNKS], F32)
            nc.vector.reciprocal(out=einv, in_=esum)
            c_sb = singles.tile([P, N_CHUNKS, OUT_CAPS], F32R)
            for n in range(N_CHUNKS):
                nc.vector.tensor_scalar_mul(
                    out=c_sb[:, n, :], in0=e_sb[:, n, :],
                    scalar1=einv[:, n : n + 1],
                )

            # ---- diagonal-selection mask: mask[o', g, o, d] = (o == o') ----
            mask = singles.tile([OUT_CAPS, GROUP, OUT_CAPS, OUT_DIM], F32)
            nc.gpsimd.memset(mask, 0.0)
            nc.gpsimd.affine_select(
                out=mask, in_=mask,
                compare_op=mybir.AluOpType.not_equal,
                fill=1.0, base=0, channel_multiplier=1,
                pattern=[[0, GROUP], [-1, OUT_CAPS], [0, OUT_DIM]],
            )

            # s[o, b, d] accumulated across all groups
            s_sb = singles.tile([OUT_CAPS, B_LOCAL, OUT_DIM], F32)

            def squash_block(b0: int, nb: int):
                """v[:, b0:b0+nb] = squash(s_sb[:, b0:b0+nb]) and DMA out."""
                s_blk = s_sb[:, b0 : b0 + nb, :]
                sq = blocks.tile([OUT_CAPS, BLK, OUT_DIM], F32, tag="sq", name="sq")[:, :nb]
                nc.vector.tensor_mul(out=sq, in0=s_blk, in1=s_blk)
                l2 = blocks.tile([OUT_CAPS, BLK], F32, tag="l2", name="l2")[:, :nb]
                nc.vector.reduce_sum(out=l2, in_=sq, axis=mybir.AxisListType.X)
                rt = blocks.tile([OUT_CAPS, BLK], F32, tag="rt", name="rt")[:, :nb]
                nc.scalar.sqrt(out=rt, in_=l2)
                denom = blocks.tile([OUT_CAPS, BLK], F32, tag="denom", name="denom")[:, :nb]
                nc.scalar.add(out=denom, in_=l2, add=1.0)
                dinv = blocks.tile([OUT_CAPS, BLK], F32, tag="dinv", name="dinv")[:, :nb]
                nc.vector.reciprocal(out=dinv, in_=denom)
                scl = blocks.tile([OUT_CAPS, BLK], F32, tag="scl", name="scl")[:, :nb]
                nc.vector.tensor_mul(out=scl, in0=rt, in1=dinv)
                # broadcast scl over d via a stride-0 AP
                scl_b = bass.AP(
                    tensor=scl.tensor, offset=scl.offset,
                    ap=[scl.ap[0], [scl.ap[1][0], nb], [0, OUT_DIM]],
                )
                v_blk = blocks.tile([OUT_CAPS, BLK, OUT_DIM], F32, tag="v_blk", name="v_blk")[:, :nb]
                nc.vector.tensor_mul(out=v_blk, in0=s_blk, in1=scl_b)
                # scalar-engine HWDGE ring: keeps the SP ring free for u loads
                nc.scalar.dma_start(out=v[:, b0 : b0 + nb, :], in_=v_blk)

            # ---- main streaming loop ----
            tile_plan = []
            tb0 = 0
            while tb0 < B_LOCAL:
                tb = min(TILE_B, B_LOCAL - tb0)
                tile_plan.append((tb0, tb))
                tb0 += tb

            last_squashed = 0
            for tb0, tb in tile_plan:
                u_tile = inp.tile([P, TILE_B, N_CHUNKS, OD], F32R)
                nc.sync.dma_start(
                    out=u_tile[:, :tb],
                    in_=u_r[:, tb0 : tb0 + tb].bitcast(F32R),
                )
                gsizes = [GROUP] * (tb // GROUP) if tb % GROUP == 0 \
                    else [2] * (tb // 2)
                g0 = 0
                for gs in gsizes:
                    b0 = tb0 + g0
                    ps = psum.tile([OUT_CAPS, GROUP, OD], F32, tag="ps", name="ps")[:, :gs]
                    for n in range(N_CHUNKS):
                        # float32r: fp32 bits, single-pass (tf32-like) matmul
                        nc.tensor.matmul(
                            ps,
                            c_sb[:, n, :],
                            u_tile[:, g0 : g0 + gs, n, :],
                            start=(n == 0), stop=(n == N_CHUNKS - 1),
                        )
                    # mask off-diagonal (o' != o) then reduce over o
                    masked = mids.tile(
                        [OUT_CAPS, GROUP, OUT_CAPS, OUT_DIM], F32,
                        tag="masked", name="masked")[:, :gs]
                    nc.vector.tensor_mul(
                        out=masked,
                        in0=ps.rearrange("q g (o d) -> q g o d", d=OUT_DIM),
                        in1=mask[:, :gs],
                    )
                    nc.vector.reduce_sum(
                        out=s_sb[:, b0 : b0 + gs, :],
                        in_=masked.rearrange("q g o d -> q g d o"),
                        axis=mybir.AxisListType.X,
                    )
                    g0 += gs
                    bend = b0 + gs
                    # squash + store finished blocks; keep the final block
                    # tiny so the post-stream tail chain is short
                    if (bend - last_squashed >= BLK or bend == B_LOCAL
                            or B_LOCAL - bend <= TILE_B):
                        squash_block(last_squashed, bend - last_squashed)
                        last_squashed = bend

    nc.compile()
    return nc


_NC_CACHE = None


def _get_program() -> bass.Bass:
    global _NC_CACHE
    if _NC_CACHE is None:
        _NC_CACHE = _build_core_program()
    return _NC_CACHE


def kernel(u_predict: np.ndarray, b: np.ndarray, n_iterations) -> np.ndarray:
    u_predict = np.ascontiguousarray(np.asarray(u_predict, dtype=np.float32))
    b = np.ascontiguousarray(np.asarray(b, dtype=np.float32))
    nc = _get_program()
    in_maps = [
        {"u": u_predict[i * B_LOCAL : (i + 1) * B_LOCAL], "b": b}
        for i in range(N_CORES)
    ]
    results = run_bass_kernel_spmd(nc, in_maps, list(range(N_CORES))).results
    # per-core v is [OUT_CAPS, B_LOCAL, OUT_DIM] -> assemble [B, OUT, DIM]
    vs = np.stack([results[i]["v"] for i in range(N_CORES)])
    out = vs.transpose(0, 2, 1, 3).reshape(B, OUT_CAPS, OUT_DIM)
    if int(n_iterations) >= 1:
        out = out[:, None]
    return np.ascontiguousarray(out.astype(np.float32))



# revision 2
# speedup vs baseline: 1.0214x; 1.0214x over previous
"""Trainium2 Bass kernel for capsule-network AgreementRouting (n_iterations=1).

Reference computation (see problem):
    c = softmax(b, axis=-1)                  # [IN, OUT] (same for every batch)
    s[b,o,d] = sum_in c[in,o] * u[b,in,o,d]  # weighted reduce over input caps
    v = squash(s)                            # per (b,o): s * l2/(1+l2)/sqrt(l2)
    out = v[:, None]                         # [B, 1, OUT, DIM]

Strategy: data-parallel over batch across 8 NeuronCores (64 batches/core).
Per core the 47 MB u-shard is streamed through SBUF in 3-batch groups with
contiguous 5760B-per-partition DMAs; each group's in-caps reduction runs on
TensorE with softmax(b) chunks as the stationary operand
(out[o', (g,o,d)] = sum_in c[in,o'] u[in,g,o,d]), the o'==o diagonal is
extracted with a precomputed identity mask + strided reduce on VectorE, and
squash + store follow immediately per group so the post-stream tail is just
the final (2-batch) group's chain.
"""

import numpy as np

import concourse.bass as bass
import concourse.tile as tile
from concourse import bacc, mybir
from concourse.bass_utils import run_bass_kernel_spmd

F32 = mybir.dt.float32
F32R = mybir.dt.float32r

B, IN_CAPS, OUT_CAPS, OUT_DIM = 512, 1152, 10, 16
N_CORES = 8
B_LOCAL = B // N_CORES            # 64 batches per core
OD = OUT_CAPS * OUT_DIM           # 160
P = 128                           # partitions
N_CHUNKS = IN_CAPS // P           # 9 contraction chunks
GROUP = 3                         # batches per PSUM accumulation group (480 cols)

# 20 groups of 3 + 2 groups of 2: the last DMAs are small so that only a
# short dependent chain remains once the final bytes land.
GROUP_SIZES = [3] * 20 + [2, 2]
assert sum(GROUP_SIZES) == B_LOCAL


def _build_core_program() -> bass.Bass:
    nc = bacc.Bacc(None)
    u = nc.dram_tensor("u", [B_LOCAL, IN_CAPS, OUT_CAPS, OUT_DIM], F32,
                       kind="ExternalInput")
    bp = nc.dram_tensor("b", [IN_CAPS, OUT_CAPS], F32, kind="ExternalInput")
    v = nc.dram_tensor("v", [OUT_CAPS, B_LOCAL, OUT_DIM], F32,
                       kind="ExternalOutput")

    # in-cap index mapping: in = p*N_CHUNKS + n (partition-major). Per (p, b)
    # the 9 chunk rows are contiguous in HBM -> 5760B runs per partition for
    # u and a single 360B run for b, keeping each DMA on one descriptor lane.
    u_r = u[:].rearrange("b (p n) o d -> p b n (o d)", p=P)
    b_r = bp[:].rearrange("(p n) o -> p n o", p=P)

    with tile.TileContext(nc) as tc:
        with (
            tc.tile_pool(name="singles", bufs=1) as singles,
            tc.tile_pool(name="inp", bufs=8) as inp,
            tc.tile_pool(name="psum", bufs=8, space="PSUM") as psum,
            tc.tile_pool(name="mids", bufs=4) as mids,
            tc.tile_pool(name="sq", bufs=4) as sqp,
        ):
            # ---- softmax over b rows: c[in, o] ----
            b_sb = singles.tile([P, N_CHUNKS, OUT_CAPS], F32)
            nc.scalar.dma_start(out=b_sb, in_=b_r)
            bmax = singles.tile([P, N_CHUNKS], F32)
            nc.vector.reduce_max(out=bmax, in_=b_sb, axis=mybir.AxisListType.X)
            negmax = singles.tile([P, N_CHUNKS], F32)
            nc.scalar.mul(out=negmax, in_=bmax, mul=-1.0)
            e_sb = singles.tile([P, N_CHUNKS, OUT_CAPS], F32)
            for n in range(N_CHUNKS):
                nc.scalar.activation(
                    out=e_sb[:, n, :], in_=b_sb[:, n, :],
                    func=mybir.ActivationFunctionType.Exp,
                    bias=negmax[:, n : n + 1], scale=1.0,
                )
            esum = singles.tile([P, N_CHUNKS], F32)
            nc.vector.reduce_sum(out=esum, in_=e_sb, axis=mybir.AxisListType.X)
            einv = singles.tile([P, N_CHUNKS], F32)
            nc.vector.reciprocal(out=einv, in_=esum)
            c_sb = singles.tile([P, N_CHUNKS, OUT_CAPS], F32R)
            for n in range(N_CHUNKS):
                nc.vector.tensor_scalar_mul(
                    out=c_sb[:, n, :], in0=e_sb[:, n, :],
                    scalar1=einv[:, n : n + 1],
                )

            # ---- diagonal-selection mask: mask[o', g, o, d] = (o == o') ----
            mask = singles.tile([OUT_CAPS, GROUP, OUT_CAPS, OUT_DIM], F32)
            nc.gpsimd.memset(mask, 0.0)
            nc.gpsimd.affine_select(
                out=mask, in_=mask,
                compare_op=mybir.AluOpType.not_equal,
                fill=1.0, base=0, channel_multiplier=1,
                pattern=[[0, GROUP], [-1, OUT_CAPS], [0, OUT_DIM]],
            )

            # ---- main streaming loop: one 3-batch group per DMA ----
            b0 = 0
            for gs in GROUP_SIZES:
                u_g = inp.tile([P, GROUP, N_CHUNKS, OD], F32R,
                               tag="u_g", name="u_g")
                nc.sync.dma_start(
                    out=u_g[:, :gs],
                    in_=u_r[:, b0 : b0 + gs].bitcast(F32R),
                )
                ps = psum.tile([OUT_CAPS, GROUP, OD], F32, tag="ps", name="ps")[:, :gs]
                for n in range(N_CHUNKS):
                    # float32r: fp32 bits, single-pass (tf32-like) matmul
                    nc.tensor.matmul(
                        ps,
                        c_sb[:, n, :],
                        u_g[:, :gs, n, :],
                        start=(n == 0), stop=(n == N_CHUNKS - 1),
                    )
                # mask off-diagonal (o' != o) then reduce over o
                masked = mids.tile(
                    [OUT_CAPS, GROUP, OUT_CAPS, OUT_DIM], F32,
                    tag="masked", name="masked")[:, :gs]
                nc.vector.tensor_mul(
                    out=masked,
                    in0=ps.rearrange("q g (o d) -> q g o d", d=OUT_DIM),
                    in1=mask[:, :gs],
                )
                s_g = sqp.tile([OUT_CAPS, GROUP, OUT_DIM], F32,
                               tag="s_g", name="s_g")[:, :gs]
                nc.vector.reduce_sum(
                    out=s_g,
                    in_=masked.rearrange("q g o d -> q g d o"),
                    axis=mybir.AxisListType.X,
                )
                # ---- squash + store for this group ----
                sq = sqp.tile([OUT_CAPS, GROUP, OUT_DIM], F32,
                              tag="sq", name="sq")[:, :gs]
                nc.vector.tensor_mul(out=sq, in0=s_g, in1=s_g)
                l2 = sqp.tile([OUT_CAPS, GROUP], F32, tag="l2", name="l2")[:, :gs]
                nc.vector.reduce_sum(out=l2, in_=sq, axis=mybir.AxisListType.X)
                rt = sqp.tile([OUT_CAPS, GROUP], F32, tag="rt", name="rt")[:, :gs]
                nc.scalar.sqrt(out=rt, in_=l2)
                denom = sqp.tile([OUT_CAPS, GROUP], F32,
                                 tag="denom", name="denom")[:, :gs]
                nc.scalar.add(out=denom, in_=l2, add=1.0)
                dinv = sqp.tile([OUT_CAPS, GROUP], F32,
                                tag="dinv", name="dinv")[:, :gs]
                nc.vector.reciprocal(out=dinv, in_=denom)
                scl = sqp.tile([OUT_CAPS, GROUP], F32, tag="scl", name="scl")[:, :gs]
                nc.vector.tensor_mul(out=scl, in0=rt, in1=dinv)
                # broadcast scl over d via a stride-0 AP
                scl_b = bass.AP(
                    tensor=scl.tensor, offset=scl.offset,
                    ap=[scl.ap[0], [scl.ap[1][0], gs], [0, OUT_DIM]],
                )
                v_g = sqp.tile([OUT_CAPS, GROUP, OUT_DIM], F32,
                               tag="v_g", name="v_g")[:, :gs]
                nc.vector.tensor_mul(out=v_g, in0=s_g, in1=scl_b)
                # scalar-engine HWDGE ring: keeps the SP ring free for u loads
                nc.scalar.dma_start(out=v[:, b0 : b0 + gs, :], in_=v_g)
                b0 += gs

    nc.compile()
    return nc


_NC_CACHE = None


def _get_program() -> bass.Bass:
    global _NC_CACHE
    if _NC_CACHE is None:
        _NC_CACHE = _build_core_program()
    return _NC_CACHE


def kernel(u_predict: np.ndarray, b: np.ndarray, n_iterations) -> np.ndarray:
    u_predict = np.ascontiguousarray(np.asarray(u_predict, dtype=np.float32))
    b = np.ascontiguousarray(np.asarray(b, dtype=np.float32))
    nc = _get_program()
    in_maps = [
        {"u": u_predict[i * B_LOCAL : (i + 1) * B_LOCAL], "b": b}
        for i in range(N_CORES)
    ]
    results = run_bass_kernel_spmd(nc, in_maps, list(range(N_CORES))).results
    # per-core v is [OUT_CAPS, B_LOCAL, OUT_DIM] -> assemble [B, OUT, DIM]
    vs = np.stack([results[i]["v"] for i in range(N_CORES)])
    out = vs.transpose(0, 2, 1, 3).reshape(B, OUT_CAPS, OUT_DIM)
    if int(n_iterations) >= 1:
        out = out[:, None]
    return np.ascontiguousarray(out.astype(np.float32))
